# revision 8
# baseline (speedup 1.0000x reference)
"""Trainium2 Bass kernel for the ConditionalDDPM forward-diffusion problem.

Computes  xt = sqrt(alpha_bar[t]) * images + sqrt(1 - alpha_bar[t]) * e
for B=65536 images of shape (1, 28, 28), t in [0, 1000).

Strategy (pure data parallel, 8 NeuronCores):
  - Shard images/e/t along batch: 8192 samples per core.
  - Pure HBM-bandwidth-bound (~358-435 GB/s/core).  The rel-err budget
    (2e-2) is exploited with reduced precision, quantized on the HOST with
    ml_dtypes (device only upconverts, exactly reproducible):
      images, e  ->  fp8 e3m4  (1 byte; measured global rel err 1.34e-2)
      out        ->  fp16      (2 bytes)
    Per-core traffic: 6.42 + 6.42 + 12.85 = 25.7MB vs 77MB in f32.
  - Per-sample scalars computed on device from t (no table gather):
    g(t) = ln(alpha_bar[t]) fitted by a degree-6 zero-intercept polynomial
    in u=(t+1)/1000 (f64 fit residual ~5e-13); a = exp(g/2), b = sqrt(1-e^g)
    in f32.  t is DMAed first on the sync queue so a/b are ready (~5us)
    before the first data tile lands.
  - Sample layout: sample s = 64*p + i lives at (partition p, unit i).
    Static SBUF tiles for x and e (no buffer recycling -> loads never wait).
  - Per unit i: scale u = a_i * x_i (ACT engine mostly, DVE for some), then
    combine out = (b_i * e_i) + u in-place into the u tile (DVE mostly,
    GPSIMD for ~24 mid-stream units - 3-way engine split so ~128 x 1us of
    elementwise work fits inside the ~61us DMA stream).
  - Stores: 4-unit groups on GPSIMD (SWDGE) for units 0-47; per-unit tail
    stores on SYNC (HWDGE) for units 48-63 to compress the end-of-stream
    dependency chain.
"""

import sys

if "/opt/trn_rl_repo" not in sys.path:
    sys.path.insert(0, "/opt/trn_rl_repo")

import numpy as np

B = 65536
T = 1000
BETA_1 = 1e-4
BETA_T = 0.02
N_CORES = 8
NS = B // N_CORES  # samples per core = 8192
PIX = 784
N_UNITS = NS // 128  # 64
LK = 8  # units per bulk load DMA
SK = 4  # units per bulk store DMA / u-tile group
BULK = 48  # units 0..47 bulk, 48..63 per-unit tail
POLY_DEG = 6

# engine split for the per-unit elementwise work.  ACT+DVE together hold
# 128 x ~1.03us of scale/combine ops, which exceeds the ~61us DMA stream,
# so GPSIMD absorbs 10 units' combines.  Pool rejects TensorScalarPtr
# (per-partition-scalar ops), so the Pool combine is decomposed into two
# tensor_tensor ops with a stride-0-broadcast scalar operand:
#   v = e * broadcast(b_i);  u = u + v
GP_COMBINE = frozenset(range(5, 45, 4))  # 10 units, early-mid stream
DVE_SCALE = frozenset(range(6, 46, 8))  # 5 units, rebalances ACT vs DVE

_cache = {}


def g_poly_coeffs() -> np.ndarray:
    """c[0..5] with g(u) ~= (((((c6*u + c5)*u + c4)*u + c3)*u + c2)*u + c1)*u,
    u = (t+1)/1000, g = ln(alpha_bar[t]).  Fit in f64; residual ~5e-13."""
    slope = (BETA_T - BETA_1) / (T - 1)
    betas = BETA_1 + slope * np.arange(T, dtype=np.float64)
    g_exact = np.cumsum(np.log1p(-betas))
    u = (np.arange(T, dtype=np.float64) + 1.0) / 1000.0
    A = np.stack([u**k for k in range(1, POLY_DEG + 1)], axis=1)
    c, *_ = np.linalg.lstsq(A, g_exact, rcond=None)
    return c


def _f8(arr32: np.ndarray) -> np.ndarray:
    import ml_dtypes

    return arr32.astype(ml_dtypes.float8_e3m4)


def build_program(ns: int = NS):
    """Build the per-core Bass program (same NEFF on all 8 cores)."""
    from concourse import bacc, mybir
    import concourse.tile as tile

    n_units = ns // 128
    f32 = mybir.dt.float32
    f16 = mybir.dt.float16
    f8 = mybir.dt.float8e3
    Alu = mybir.AluOpType
    Act = mybir.ActivationFunctionType
    coeffs = [float(c) for c in g_poly_coeffs()]

    nc = bacc.Bacc(
        "TRN2",
        target_bir_lowering=False,
        debug=False,
        enable_asserts=False,
        num_devices=N_CORES,
    )
    x = nc.dram_tensor("x", [ns, PIX], f8, kind="ExternalInput").ap()
    y = nc.dram_tensor("y", [ns, PIX], f8, kind="ExternalInput").ap()
    tt = nc.dram_tensor("t", [ns], mybir.dt.int32, kind="ExternalInput").ap()
    out = nc.dram_tensor("out", [ns, PIX], f16, kind="ExternalOutput").ap()

    # sample s = 64*p + i  ->  (partition p, unit i)
    x_v = x.rearrange("(p i) m -> p i m", p=128)
    y_v = y.rearrange("(p i) m -> p i m", p=128)
    o_v = out.rearrange("(p i) m -> p i m", p=128)
    t_v = tt.rearrange("(p i) -> p i", p=128)  # contiguous 256B per partition

    def scale_unit(i, out_ap, in_ap, a_t):
        if i in DVE_SCALE:
            nc.vector.tensor_scalar_mul(
                out=out_ap, in0=in_ap, scalar1=a_t[:, i : i + 1]
            )
        else:
            nc.scalar.activation(
                out=out_ap, in_=in_ap, func=Act.Copy, scale=a_t[:, i : i + 1]
            )

    from concourse.bass import broadcast_tensor_aps

    def combine_unit(i, u_ap, e_ap, b_t, vpool=None):
        if i in GP_COMBINE:
            # Pool can't run per-partition-scalar ops; use two tensor_tensor
            # ops with the b_i column broadcast along the free dim instead.
            v = vpool.tile([128, PIX], f16)
            e_b, b_b = broadcast_tensor_aps(e_ap, b_t[:, i : i + 1])
            nc.gpsimd.tensor_tensor(out=v[:], in0=e_b, in1=b_b, op=Alu.mult)
            nc.gpsimd.tensor_tensor(out=u_ap, in0=u_ap, in1=v[:], op=Alu.add)
            return
        nc.vector.scalar_tensor_tensor(
            out=u_ap,
            in0=e_ap,
            scalar=b_t[:, i : i + 1],
            in1=u_ap,
            op0=Alu.mult,
            op1=Alu.add,
        )

    with tile.TileContext(nc) as tc:
        with (
            tc.tile_pool(name="xs", bufs=1) as xpool,
            tc.tile_pool(name="ys", bufs=1) as ypool,
            tc.tile_pool(name="us", bufs=9) as upool,
            tc.tile_pool(name="ut", bufs=8) as utail,
            tc.tile_pool(name="vs", bufs=4) as vpool,
            tc.tile_pool(name="singles", bufs=1) as singles,
        ):
            # ---- t load first (sync queue) so scalars are ready early ----
            ti = singles.tile([128, n_units], mybir.dt.int32)
            nc.sync.dma_start(out=ti[:], in_=t_v)

            # ---- per-sample scalars: a = exp(g/2), b = sqrt(1 - exp(g)) ----
            # u = (t + 1) / 1000   (int32 in, f32 out)
            uu = singles.tile([128, n_units], f32)
            nc.vector.tensor_scalar(
                out=uu[:], in0=ti[:], scalar1=1.0, scalar2=0.001,
                op0=Alu.add, op1=Alu.mult,
            )
            # Horner with zero intercept: h = u*c6; h = (h + c_k)*u, k=5..1
            hh = singles.tile([128, n_units], f32)
            nc.vector.tensor_scalar_mul(out=hh[:], in0=uu[:], scalar1=coeffs[5])
            for kk_ in range(POLY_DEG - 2, -1, -1):
                nc.vector.scalar_tensor_tensor(
                    out=hh[:], in0=hh[:], scalar=coeffs[kk_], in1=uu[:],
                    op0=Alu.add, op1=Alu.mult,
                )
            # a = exp(0.5*g)
            a_t = singles.tile([128, n_units], f32)
            nc.scalar.activation(out=a_t[:], in_=hh[:], func=Act.Exp, scale=0.5)
            # b = sqrt(1 - exp(g))
            b_t = singles.tile([128, n_units], f32)
            nc.scalar.activation(out=b_t[:], in_=hh[:], func=Act.Exp)
            nc.vector.tensor_scalar(
                out=b_t[:], in0=b_t[:], scalar1=1.0, scalar2=-1.0,
                op0=Alu.subtract, op1=Alu.mult,
            )
            nc.scalar.activation(out=b_t[:], in_=b_t[:], func=Act.Sqrt)

            # ---- static input tiles: loads never wait on anything ----
            x_sb = xpool.tile([128, n_units, PIX], f8)
            e_sb = ypool.tile([128, n_units, PIX], f8)
            # bulk loads, LK units per DMA (x/e interleaved in unit order)
            for c in range(BULK // LK):
                i0 = c * LK
                nc.sync.dma_start(out=x_sb[:, i0 : i0 + LK, :], in_=x_v[:, i0 : i0 + LK, :])
                nc.sync.dma_start(out=e_sb[:, i0 : i0 + LK, :], in_=y_v[:, i0 : i0 + LK, :])
            # per-unit tail loads
            for i in range(BULK, n_units):
                nc.sync.dma_start(out=x_sb[:, i, :], in_=x_v[:, i, :])
                nc.sync.dma_start(out=e_sb[:, i, :], in_=y_v[:, i, :])

            # ---- bulk compute + stores: SK-unit u tiles, stores on gpsimd ----
            for g in range(BULK // SK):
                i0 = g * SK
                u4 = upool.tile([128, SK, PIX], f16)
                for kk in range(SK):
                    i = i0 + kk
                    scale_unit(i, u4[:, kk, :], x_sb[:, i, :], a_t)
                    combine_unit(i, u4[:, kk, :], e_sb[:, i, :], b_t, vpool)
                nc.gpsimd.dma_start(out=o_v[:, i0 : i0 + SK, :], in_=u4[:])

            # ---- tail: per-unit compute + per-unit stores on sync ----
            for i in range(BULK, n_units):
                u1 = utail.tile([128, PIX], f16)
                scale_unit(i, u1[:], x_sb[:, i, :], a_t)
                combine_unit(i, u1[:], e_sb[:, i, :], b_t, vpool)
                nc.sync.dma_start(out=o_v[:, i, :], in_=u1[:])

    nc.compile()
    return nc


def make_in_maps(images, e, t):
    x = _f8(np.asarray(images, dtype=np.float32).reshape(B, PIX))
    yy = _f8(np.asarray(e, dtype=np.float32).reshape(B, PIX))
    tt = np.ascontiguousarray(np.asarray(t, dtype=np.int32).reshape(B))
    in_maps = []
    for c in range(N_CORES):
        sl = slice(c * NS, (c + 1) * NS)
        in_maps.append(
            {
                "x": np.ascontiguousarray(x[sl]),
                "y": np.ascontiguousarray(yy[sl]),
                "t": np.ascontiguousarray(tt[sl]),
            }
        )
    return in_maps


def _get_runner():
    """Build (once) a jitted shard_map callable over the 8 cores.

    Mirrors concourse.bass2jax.run_bass_via_pjrt, but caches the compiled
    executable so repeated kernel() calls skip retracing/recompiling, and
    keeps the output placeholder buffers resident on device.
    """
    if "runner" in _cache:
        return _cache["runner"]

    import jax
    from jax.sharding import Mesh, PartitionSpec, NamedSharding
    from jax.experimental.shard_map import shard_map
    from concourse import mybir
    from concourse.bass2jax import (
        _bass_exec_p,
        install_neuronx_cc_hook,
        partition_id_tensor,
    )

    nc = _cache.get("nc")
    if nc is None:
        nc = _cache["nc"] = build_program()

    install_neuronx_cc_hook()

    partition_name = nc.partition_id_tensor.name if nc.partition_id_tensor else None
    in_names, out_names, out_avals = [], [], []
    for alloc in nc.m.functions[0].allocations:
        if not isinstance(alloc, mybir.MemoryLocationSet):
            continue
        name = alloc.memorylocations[0].name
        if alloc.kind == "ExternalInput":
            if name != partition_name:
                in_names.append(name)
        elif alloc.kind == "ExternalOutput":
            out_names.append(name)
            out_avals.append(
                jax.core.ShapedArray(tuple(alloc.tensor_shape), mybir.dt.np(alloc.dtype))
            )
    n_params = len(in_names)
    all_names = list(in_names) + out_names
    if partition_name is not None:
        all_names.append(partition_name)

    def _body(*args):
        # args = params + output placeholder buffers (the hook's parameter-
        # order check requires every bass_exec operand to be a jit parameter)
        operands = list(args)
        if partition_name is not None:
            operands.append(partition_id_tensor())
        outs = _bass_exec_p.bind(
            *operands,
            out_avals=tuple(out_avals),
            in_names=tuple(all_names),
            out_names=tuple(out_names),
            lowering_input_output_aliases=(),
            sim_require_finite=True,
            sim_require_nnan=True,
            nc=nc,
        )
        return tuple(outs)

    devices = jax.devices()[:N_CORES]
    assert len(devices) == N_CORES
    mesh = Mesh(np.asarray(devices), ("core",))
    n_outs = len(out_names)
    sharded = jax.jit(
        shard_map(
            _body,
            mesh=mesh,
            in_specs=(PartitionSpec("core"),) * (n_params + n_outs),
            out_specs=(PartitionSpec("core"),) * n_outs,
            check_rep=False,
        ),
        keep_unused=True,
    )
    # Output placeholder buffers: uploaded to device once, NOT donated, so
    # they stay valid and cost nothing on subsequent calls.
    zeros_dev = [
        jax.device_put(
            np.zeros((N_CORES * a.shape[0], *a.shape[1:]), a.dtype),
            NamedSharding(mesh, PartitionSpec("core")),
        )
        for a in out_avals
    ]
    _cache["runner"] = (sharded, in_names, out_names, zeros_dev)
    return _cache["runner"]


def kernel(images, e, t):
    images = np.asarray(images)
    orig_shape = images.shape

    x = _f8(images.astype(np.float32, copy=False).reshape(B, PIX))
    yy = _f8(np.asarray(e, dtype=np.float32).reshape(B, PIX))
    tt = np.ascontiguousarray(np.asarray(t, dtype=np.int32).reshape(B))

    try:
        sharded, in_names, out_names, zeros_dev = _get_runner()
        global_in = {"x": x, "y": yy, "t": tt}
        out_arrs = sharded(*[global_in[n] for n in in_names], *zeros_dev)
        full = np.asarray(out_arrs[out_names.index("out")])
    except Exception:
        # Fallback: the stock (slower, but battle-tested) execution path.
        from concourse import bass_utils

        if "nc" not in _cache:
            _cache["nc"] = build_program()
        res = bass_utils.run_bass_kernel_spmd(
            _cache["nc"], make_in_maps(images, e, t), core_ids=list(range(N_CORES))
        )
        full = np.concatenate([res.results[c]["out"] for c in range(N_CORES)], axis=0)

    return full.astype(np.float32).reshape(orig_shape)


# revision 11
# speedup vs baseline: 1.3270x; 1.3270x over previous
"""Trainium2 Bass kernel for the ConditionalDDPM forward-diffusion problem.

Computes  xt = sqrt(alpha_bar[t]) * images + sqrt(1 - alpha_bar[t]) * e
for B=65536 images of shape (1, 28, 28), t in [0, 1000).

Strategy (pure data parallel, 8 NeuronCores):
  - Shard images/e/t along batch: 8192 samples per core.
  - Pure HBM-bandwidth-bound (~358-435 GB/s/core).  The rel-err budget
    (2e-2) is exploited with reduced precision, quantized on the HOST with
    ml_dtypes (device only upconverts, exactly reproducible):
      images, e  ->  fp8 e3m4  (1 byte; measured global rel err 1.34e-2)
      out        ->  fp16      (2 bytes)
    Per-core traffic: 6.42 + 6.42 + 12.85 = 25.7MB vs 77MB in f32.
  - Per-sample scalars computed on device from t (no table gather):
    g(t) = ln(alpha_bar[t]) fitted by a degree-6 zero-intercept polynomial
    in u=(t+1)/1000 (f64 fit residual ~5e-13); a = exp(g/2), b = sqrt(1-e^g)
    in f32.  t is DMAed first on the sync queue so a/b are ready (~5us)
    before the first data tile lands.
  - Sample layout: sample s = 64*p + i lives at (partition p, unit i).
    Static SBUF tiles for x and e (no buffer recycling -> loads never wait).
  - Per unit i: scale u = a_i * x_i (ACT engine), then combine
    out = (b_i * e_i) + u in-place into the u tile (DVE).  The 2x64 ops at
    ~1.03us each make the kernel compute-bound (~66us/engine vs ~61us DMA
    stream).  GPSIMD elementwise offload was measured and REJECTED: Q7
    compute slows concurrent DVE ops 1.55x via SBUF port contention.
  - Stores: 4-unit groups on GPSIMD (SWDGE triggers only - harmless) for
    units 0-47; per-unit tail stores on SYNC (HWDGE) for units 48-63 to
    compress the end-of-stream dependency chain.
  - u-tile pool is kept at bufs=6: a bufs=9 variant measured 81.5us but
    intermittently corrupted ~2 units' output on the pjrt path (race);
    bufs=6 is the verified-correct configuration at 93.2us.
"""

import sys

if "/opt/trn_rl_repo" not in sys.path:
    sys.path.insert(0, "/opt/trn_rl_repo")

import numpy as np

B = 65536
T = 1000
BETA_1 = 1e-4
BETA_T = 0.02
N_CORES = 8
NS = B // N_CORES  # samples per core = 8192
PIX = 784
N_UNITS = NS // 128  # 64
LK = 8  # units per bulk load DMA
SK = 4  # units per bulk store DMA / u-tile group
BULK = 48  # units 0..47 bulk, 48..63 per-unit tail
POLY_DEG = 6

# engine split for the per-unit elementwise work: ACT does all 64 scale
# ops, DVE all 64 combines (Pool rejects TensorScalarPtr, so no gpsimd
# compute; gpsimd only triggers the bulk stores)
GP_COMBINE = frozenset()
DVE_SCALE = frozenset()

_cache = {}


def g_poly_coeffs() -> np.ndarray:
    """c[0..5] with g(u) ~= (((((c6*u + c5)*u + c4)*u + c3)*u + c2)*u + c1)*u,
    u = (t+1)/1000, g = ln(alpha_bar[t]).  Fit in f64; residual ~5e-13."""
    slope = (BETA_T - BETA_1) / (T - 1)
    betas = BETA_1 + slope * np.arange(T, dtype=np.float64)
    g_exact = np.cumsum(np.log1p(-betas))
    u = (np.arange(T, dtype=np.float64) + 1.0) / 1000.0
    A = np.stack([u**k for k in range(1, POLY_DEG + 1)], axis=1)
    c, *_ = np.linalg.lstsq(A, g_exact, rcond=None)
    return c


def _f8(arr32: np.ndarray) -> np.ndarray:
    import ml_dtypes

    return arr32.astype(ml_dtypes.float8_e3m4)


def build_program(ns: int = NS):
    """Build the per-core Bass program (same NEFF on all 8 cores)."""
    from concourse import bacc, mybir
    import concourse.tile as tile

    n_units = ns // 128
    f32 = mybir.dt.float32
    f16 = mybir.dt.float16
    f8 = mybir.dt.float8e3
    Alu = mybir.AluOpType
    Act = mybir.ActivationFunctionType
    coeffs = [float(c) for c in g_poly_coeffs()]

    nc = bacc.Bacc(
        "TRN2",
        target_bir_lowering=False,
        debug=False,
        enable_asserts=False,
        num_devices=N_CORES,
    )
    x = nc.dram_tensor("x", [ns, PIX], f8, kind="ExternalInput").ap()
    y = nc.dram_tensor("y", [ns, PIX], f8, kind="ExternalInput").ap()
    tt = nc.dram_tensor("t", [ns], mybir.dt.int32, kind="ExternalInput").ap()
    out = nc.dram_tensor("out", [ns, PIX], f16, kind="ExternalOutput").ap()

    # sample s = 64*p + i  ->  (partition p, unit i)
    x_v = x.rearrange("(p i) m -> p i m", p=128)
    y_v = y.rearrange("(p i) m -> p i m", p=128)
    o_v = out.rearrange("(p i) m -> p i m", p=128)
    t_v = tt.rearrange("(p i) -> p i", p=128)  # contiguous 256B per partition

    def scale_unit(i, out_ap, in_ap, a_t):
        if i in DVE_SCALE:
            nc.vector.tensor_scalar_mul(
                out=out_ap, in0=in_ap, scalar1=a_t[:, i : i + 1]
            )
        else:
            nc.scalar.activation(
                out=out_ap, in_=in_ap, func=Act.Copy, scale=a_t[:, i : i + 1]
            )

    def combine_unit(i, u_ap, e_ap, b_t):
        eng = nc.gpsimd if i in GP_COMBINE else nc.vector
        eng.scalar_tensor_tensor(
            out=u_ap,
            in0=e_ap,
            scalar=b_t[:, i : i + 1],
            in1=u_ap,
            op0=Alu.mult,
            op1=Alu.add,
        )

    with tile.TileContext(nc) as tc:
        with (
            tc.tile_pool(name="xs", bufs=1) as xpool,
            tc.tile_pool(name="ys", bufs=1) as ypool,
            tc.tile_pool(name="us", bufs=6) as upool,
            tc.tile_pool(name="ut", bufs=8) as utail,
            tc.tile_pool(name="singles", bufs=1) as singles,
        ):
            # ---- t load first (sync queue) so scalars are ready early ----
            ti = singles.tile([128, n_units], mybir.dt.int32)
            nc.sync.dma_start(out=ti[:], in_=t_v)

            # ---- per-sample scalars: a = exp(g/2), b = sqrt(1 - exp(g)) ----
            # u = (t + 1) / 1000   (int32 in, f32 out)
            uu = singles.tile([128, n_units], f32)
            nc.vector.tensor_scalar(
                out=uu[:], in0=ti[:], scalar1=1.0, scalar2=0.001,
                op0=Alu.add, op1=Alu.mult,
            )
            # Horner with zero intercept: h = u*c6; h = (h + c_k)*u, k=5..1
            hh = singles.tile([128, n_units], f32)
            nc.vector.tensor_scalar_mul(out=hh[:], in0=uu[:], scalar1=coeffs[5])
            for kk_ in range(POLY_DEG - 2, -1, -1):
                nc.vector.scalar_tensor_tensor(
                    out=hh[:], in0=hh[:], scalar=coeffs[kk_], in1=uu[:],
                    op0=Alu.add, op1=Alu.mult,
                )
            # a = exp(0.5*g)
            a_t = singles.tile([128, n_units], f32)
            nc.scalar.activation(out=a_t[:], in_=hh[:], func=Act.Exp, scale=0.5)
            # b = sqrt(1 - exp(g))
            b_t = singles.tile([128, n_units], f32)
            nc.scalar.activation(out=b_t[:], in_=hh[:], func=Act.Exp)
            nc.vector.tensor_scalar(
                out=b_t[:], in0=b_t[:], scalar1=1.0, scalar2=-1.0,
                op0=Alu.subtract, op1=Alu.mult,
            )
            nc.scalar.activation(out=b_t[:], in_=b_t[:], func=Act.Sqrt)

            # ---- static input tiles: loads never wait on anything ----
            x_sb = xpool.tile([128, n_units, PIX], f8)
            e_sb = ypool.tile([128, n_units, PIX], f8)
            # bulk loads, LK units per DMA (x/e interleaved in unit order)
            for c in range(BULK // LK):
                i0 = c * LK
                nc.sync.dma_start(out=x_sb[:, i0 : i0 + LK, :], in_=x_v[:, i0 : i0 + LK, :])
                nc.sync.dma_start(out=e_sb[:, i0 : i0 + LK, :], in_=y_v[:, i0 : i0 + LK, :])
            # per-unit tail loads
            for i in range(BULK, n_units):
                nc.sync.dma_start(out=x_sb[:, i, :], in_=x_v[:, i, :])
                nc.sync.dma_start(out=e_sb[:, i, :], in_=y_v[:, i, :])

            # ---- bulk compute + stores: SK-unit u tiles, stores on gpsimd ----
            for g in range(BULK // SK):
                i0 = g * SK
                u4 = upool.tile([128, SK, PIX], f16)
                for kk in range(SK):
                    i = i0 + kk
                    scale_unit(i, u4[:, kk, :], x_sb[:, i, :], a_t)
                    combine_unit(i, u4[:, kk, :], e_sb[:, i, :], b_t)
                nc.gpsimd.dma_start(out=o_v[:, i0 : i0 + SK, :], in_=u4[:])

            # ---- tail: per-unit compute + per-unit stores on sync ----
            for i in range(BULK, n_units):
                u1 = utail.tile([128, PIX], f16)
                scale_unit(i, u1[:], x_sb[:, i, :], a_t)
                combine_unit(i, u1[:], e_sb[:, i, :], b_t)
                nc.sync.dma_start(out=o_v[:, i, :], in_=u1[:])

    nc.compile()
    return nc


def make_in_maps(images, e, t):
    x = _f8(np.asarray(images, dtype=np.float32).reshape(B, PIX))
    yy = _f8(np.asarray(e, dtype=np.float32).reshape(B, PIX))
    tt = np.ascontiguousarray(np.asarray(t, dtype=np.int32).reshape(B))
    in_maps = []
    for c in range(N_CORES):
        sl = slice(c * NS, (c + 1) * NS)
        in_maps.append(
            {
                "x": np.ascontiguousarray(x[sl]),
                "y": np.ascontiguousarray(yy[sl]),
                "t": np.ascontiguousarray(tt[sl]),
            }
        )
    return in_maps


def _get_runner():
    """Build (once) a jitted shard_map callable over the 8 cores.

    Mirrors concourse.bass2jax.run_bass_via_pjrt, but caches the compiled
    executable so repeated kernel() calls skip retracing/recompiling, and
    keeps the output placeholder buffers resident on device.
    """
    if "runner" in _cache:
        return _cache["runner"]

    import jax
    from jax.sharding import Mesh, PartitionSpec, NamedSharding
    from jax.experimental.shard_map import shard_map
    from concourse import mybir
    from concourse.bass2jax import (
        _bass_exec_p,
        install_neuronx_cc_hook,
        partition_id_tensor,
    )

    nc = _cache.get("nc")
    if nc is None:
        nc = _cache["nc"] = build_program()

    install_neuronx_cc_hook()

    partition_name = nc.partition_id_tensor.name if nc.partition_id_tensor else None
    in_names, out_names, out_avals = [], [], []
    for alloc in nc.m.functions[0].allocations:
        if not isinstance(alloc, mybir.MemoryLocationSet):
            continue
        name = alloc.memorylocations[0].name
        if alloc.kind == "ExternalInput":
            if name != partition_name:
                in_names.append(name)
        elif alloc.kind == "ExternalOutput":
            out_names.append(name)
            out_avals.append(
                jax.core.ShapedArray(tuple(alloc.tensor_shape), mybir.dt.np(alloc.dtype))
            )
    n_params = len(in_names)
    all_names = list(in_names) + out_names
    if partition_name is not None:
        all_names.append(partition_name)

    def _body(*args):
        # args = params + output placeholder buffers (the hook's parameter-
        # order check requires every bass_exec operand to be a jit parameter)
        operands = list(args)
        if partition_name is not None:
            operands.append(partition_id_tensor())
        outs = _bass_exec_p.bind(
            *operands,
            out_avals=tuple(out_avals),
            in_names=tuple(all_names),
            out_names=tuple(out_names),
            lowering_input_output_aliases=(),
            sim_require_finite=True,
            sim_require_nnan=True,
            nc=nc,
        )
        return tuple(outs)

    devices = jax.devices()[:N_CORES]
    assert len(devices) == N_CORES
    mesh = Mesh(np.asarray(devices), ("core",))
    n_outs = len(out_names)
    sharded = jax.jit(
        shard_map(
            _body,
            mesh=mesh,
            in_specs=(PartitionSpec("core"),) * (n_params + n_outs),
            out_specs=(PartitionSpec("core"),) * n_outs,
            check_rep=False,
        ),
        keep_unused=True,
    )
    # Output placeholder buffers: uploaded to device once, NOT donated, so
    # they stay valid and cost nothing on subsequent calls.
    zeros_dev = [
        jax.device_put(
            np.zeros((N_CORES * a.shape[0], *a.shape[1:]), a.dtype),
            NamedSharding(mesh, PartitionSpec("core")),
        )
        for a in out_avals
    ]
    _cache["runner"] = (sharded, in_names, out_names, zeros_dev)
    return _cache["runner"]


def kernel(images, e, t):
    images = np.asarray(images)
    orig_shape = images.shape

    x = _f8(images.astype(np.float32, copy=False).reshape(B, PIX))
    yy = _f8(np.asarray(e, dtype=np.float32).reshape(B, PIX))
    tt = np.ascontiguousarray(np.asarray(t, dtype=np.int32).reshape(B))

    try:
        sharded, in_names, out_names, zeros_dev = _get_runner()
        global_in = {"x": x, "y": yy, "t": tt}
        out_arrs = sharded(*[global_in[n] for n in in_names], *zeros_dev)
        full = np.asarray(out_arrs[out_names.index("out")])
    except Exception:
        # Fallback: the stock (slower, but battle-tested) execution path.
        from concourse import bass_utils

        if "nc" not in _cache:
            _cache["nc"] = build_program()
        res = bass_utils.run_bass_kernel_spmd(
            _cache["nc"], make_in_maps(images, e, t), core_ids=list(range(N_CORES))
        )
        full = np.concatenate([res.results[c]["out"] for c in range(N_CORES)], axis=0)

    return full.astype(np.float32).reshape(orig_shape)


# revision 12
# speedup vs baseline: 1.4263x; 1.0748x over previous
"""Trainium2 Bass kernel for the ConditionalDDPM forward-diffusion problem.

Computes  xt = sqrt(alpha_bar[t]) * images + sqrt(1 - alpha_bar[t]) * e
for B=65536 images of shape (1, 28, 28), t in [0, 1000).

Strategy (pure data parallel, 8 NeuronCores):
  - Shard images/e/t along batch: 8192 samples per core.
  - Pure HBM-bandwidth-bound (~358-435 GB/s/core).  The rel-err budget
    (2e-2) is exploited with reduced precision, quantized on the HOST with
    ml_dtypes (device only upconverts, exactly reproducible):
      images, e  ->  fp8 e3m4  (1 byte; measured global rel err 1.34e-2)
      out        ->  fp16      (2 bytes)
    Per-core traffic: 6.42 + 6.42 + 12.85 = 25.7MB vs 77MB in f32.
  - Per-sample scalars computed on device from t (no table gather):
    g(t) = ln(alpha_bar[t]) fitted by a degree-6 zero-intercept polynomial
    in u=(t+1)/1000 (f64 fit residual ~5e-13); a = exp(g/2), b = sqrt(1-e^g)
    in f32.  t is DMAed first on the sync queue so a/b are ready (~5us)
    before the first data tile lands.
  - Sample layout: sample s = 64*p + i lives at (partition p, unit i).
    Static SBUF tiles for x and e (no buffer recycling -> loads never wait).
  - Per unit i: scale u = a_i * x_i (ACT engine), then combine
    out = (b_i * e_i) + u in-place into the u tile (DVE).  The 2x64 ops at
    ~1.03us each make the kernel compute-bound (~66us/engine vs ~61us DMA
    stream).  GPSIMD elementwise offload was measured and REJECTED: Q7
    compute slows concurrent DVE ops 1.55x via SBUF port contention.
  - ALL stores on SYNC (HWDGE): 4-unit groups for units 0-47, per-unit
    tail stores for units 48-63.  An earlier variant with bulk stores on
    GPSIMD (SWDGE) intermittently corrupted ~2 units when the u-tile WAR
    window was tight (bufs=9); the HWDGE store path has never corrupted.
  - u-tile pool bufs=12: stores lag compute by ~12-15us, so the
    write-after-read recycle dependency (scale of group g+12 waits store
    of group g, ~25us of slack) never stalls the compute engines.
"""

import sys

if "/opt/trn_rl_repo" not in sys.path:
    sys.path.insert(0, "/opt/trn_rl_repo")

import numpy as np

B = 65536
T = 1000
BETA_1 = 1e-4
BETA_T = 0.02
N_CORES = 8
NS = B // N_CORES  # samples per core = 8192
PIX = 784
N_UNITS = NS // 128  # 64
LK = 8  # units per bulk load DMA
SK = 4  # units per bulk store DMA / u-tile group
BULK = 48  # units 0..47 bulk, 48..63 per-unit tail
POLY_DEG = 6

# engine split for the per-unit elementwise work: ACT does all 64 scale
# ops, DVE all 64 combines (Pool rejects TensorScalarPtr, so no gpsimd
# compute; gpsimd only triggers the bulk stores)
GP_COMBINE = frozenset()
DVE_SCALE = frozenset()

_cache = {}


def g_poly_coeffs() -> np.ndarray:
    """c[0..5] with g(u) ~= (((((c6*u + c5)*u + c4)*u + c3)*u + c2)*u + c1)*u,
    u = (t+1)/1000, g = ln(alpha_bar[t]).  Fit in f64; residual ~5e-13."""
    slope = (BETA_T - BETA_1) / (T - 1)
    betas = BETA_1 + slope * np.arange(T, dtype=np.float64)
    g_exact = np.cumsum(np.log1p(-betas))
    u = (np.arange(T, dtype=np.float64) + 1.0) / 1000.0
    A = np.stack([u**k for k in range(1, POLY_DEG + 1)], axis=1)
    c, *_ = np.linalg.lstsq(A, g_exact, rcond=None)
    return c


def _f8(arr32: np.ndarray) -> np.ndarray:
    import ml_dtypes

    return arr32.astype(ml_dtypes.float8_e3m4)


def build_program(ns: int = NS):
    """Build the per-core Bass program (same NEFF on all 8 cores)."""
    from concourse import bacc, mybir
    import concourse.tile as tile

    n_units = ns // 128
    f32 = mybir.dt.float32
    f16 = mybir.dt.float16
    f8 = mybir.dt.float8e3
    Alu = mybir.AluOpType
    Act = mybir.ActivationFunctionType
    coeffs = [float(c) for c in g_poly_coeffs()]

    nc = bacc.Bacc(
        "TRN2",
        target_bir_lowering=False,
        debug=False,
        enable_asserts=False,
        num_devices=N_CORES,
    )
    x = nc.dram_tensor("x", [ns, PIX], f8, kind="ExternalInput").ap()
    y = nc.dram_tensor("y", [ns, PIX], f8, kind="ExternalInput").ap()
    tt = nc.dram_tensor("t", [ns], mybir.dt.int32, kind="ExternalInput").ap()
    out = nc.dram_tensor("out", [ns, PIX], f16, kind="ExternalOutput").ap()

    # sample s = 64*p + i  ->  (partition p, unit i)
    x_v = x.rearrange("(p i) m -> p i m", p=128)
    y_v = y.rearrange("(p i) m -> p i m", p=128)
    o_v = out.rearrange("(p i) m -> p i m", p=128)
    t_v = tt.rearrange("(p i) -> p i", p=128)  # contiguous 256B per partition

    def scale_unit(i, out_ap, in_ap, a_t):
        if i in DVE_SCALE:
            nc.vector.tensor_scalar_mul(
                out=out_ap, in0=in_ap, scalar1=a_t[:, i : i + 1]
            )
        else:
            nc.scalar.activation(
                out=out_ap, in_=in_ap, func=Act.Copy, scale=a_t[:, i : i + 1]
            )

    def combine_unit(i, u_ap, e_ap, b_t):
        eng = nc.gpsimd if i in GP_COMBINE else nc.vector
        eng.scalar_tensor_tensor(
            out=u_ap,
            in0=e_ap,
            scalar=b_t[:, i : i + 1],
            in1=u_ap,
            op0=Alu.mult,
            op1=Alu.add,
        )

    with tile.TileContext(nc) as tc:
        with (
            tc.tile_pool(name="xs", bufs=1) as xpool,
            tc.tile_pool(name="ys", bufs=1) as ypool,
            tc.tile_pool(name="us", bufs=12) as upool,
            tc.tile_pool(name="ut", bufs=8) as utail,
            tc.tile_pool(name="singles", bufs=1) as singles,
        ):
            # ---- t load first (sync queue) so scalars are ready early ----
            ti = singles.tile([128, n_units], mybir.dt.int32)
            nc.sync.dma_start(out=ti[:], in_=t_v)

            # ---- per-sample scalars: a = exp(g/2), b = sqrt(1 - exp(g)) ----
            # u = (t + 1) / 1000   (int32 in, f32 out)
            uu = singles.tile([128, n_units], f32)
            nc.vector.tensor_scalar(
                out=uu[:], in0=ti[:], scalar1=1.0, scalar2=0.001,
                op0=Alu.add, op1=Alu.mult,
            )
            # Horner with zero intercept: h = u*c6; h = (h + c_k)*u, k=5..1
            hh = singles.tile([128, n_units], f32)
            nc.vector.tensor_scalar_mul(out=hh[:], in0=uu[:], scalar1=coeffs[5])
            for kk_ in range(POLY_DEG - 2, -1, -1):
                nc.vector.scalar_tensor_tensor(
                    out=hh[:], in0=hh[:], scalar=coeffs[kk_], in1=uu[:],
                    op0=Alu.add, op1=Alu.mult,
                )
            # a = exp(0.5*g)
            a_t = singles.tile([128, n_units], f32)
            nc.scalar.activation(out=a_t[:], in_=hh[:], func=Act.Exp, scale=0.5)
            # b = sqrt(1 - exp(g))
            b_t = singles.tile([128, n_units], f32)
            nc.scalar.activation(out=b_t[:], in_=hh[:], func=Act.Exp)
            nc.vector.tensor_scalar(
                out=b_t[:], in0=b_t[:], scalar1=1.0, scalar2=-1.0,
                op0=Alu.subtract, op1=Alu.mult,
            )
            nc.scalar.activation(out=b_t[:], in_=b_t[:], func=Act.Sqrt)

            # ---- static input tiles: loads never wait on anything ----
            x_sb = xpool.tile([128, n_units, PIX], f8)
            e_sb = ypool.tile([128, n_units, PIX], f8)
            # bulk loads, LK units per DMA (x/e interleaved in unit order)
            for c in range(BULK // LK):
                i0 = c * LK
                nc.sync.dma_start(out=x_sb[:, i0 : i0 + LK, :], in_=x_v[:, i0 : i0 + LK, :])
                nc.sync.dma_start(out=e_sb[:, i0 : i0 + LK, :], in_=y_v[:, i0 : i0 + LK, :])
            # per-unit tail loads
            for i in range(BULK, n_units):
                nc.sync.dma_start(out=x_sb[:, i, :], in_=x_v[:, i, :])
                nc.sync.dma_start(out=e_sb[:, i, :], in_=y_v[:, i, :])

            # ---- bulk compute + stores: SK-unit u tiles, stores on gpsimd ----
            for g in range(BULK // SK):
                i0 = g * SK
                u4 = upool.tile([128, SK, PIX], f16)
                for kk in range(SK):
                    i = i0 + kk
                    scale_unit(i, u4[:, kk, :], x_sb[:, i, :], a_t)
                    combine_unit(i, u4[:, kk, :], e_sb[:, i, :], b_t)
                nc.sync.dma_start(out=o_v[:, i0 : i0 + SK, :], in_=u4[:])

            # ---- tail: per-unit compute + per-unit stores on sync ----
            for i in range(BULK, n_units):
                u1 = utail.tile([128, PIX], f16)
                scale_unit(i, u1[:], x_sb[:, i, :], a_t)
                combine_unit(i, u1[:], e_sb[:, i, :], b_t)
                nc.sync.dma_start(out=o_v[:, i, :], in_=u1[:])

    nc.compile()
    return nc


def make_in_maps(images, e, t):
    x = _f8(np.asarray(images, dtype=np.float32).reshape(B, PIX))
    yy = _f8(np.asarray(e, dtype=np.float32).reshape(B, PIX))
    tt = np.ascontiguousarray(np.asarray(t, dtype=np.int32).reshape(B))
    in_maps = []
    for c in range(N_CORES):
        sl = slice(c * NS, (c + 1) * NS)
        in_maps.append(
            {
                "x": np.ascontiguousarray(x[sl]),
                "y": np.ascontiguousarray(yy[sl]),
                "t": np.ascontiguousarray(tt[sl]),
            }
        )
    return in_maps


def _get_runner():
    """Build (once) a jitted shard_map callable over the 8 cores.

    Mirrors concourse.bass2jax.run_bass_via_pjrt, but caches the compiled
    executable so repeated kernel() calls skip retracing/recompiling, and
    keeps the output placeholder buffers resident on device.
    """
    if "runner" in _cache:
        return _cache["runner"]

    import jax
    from jax.sharding import Mesh, PartitionSpec, NamedSharding
    from jax.experimental.shard_map import shard_map
    from concourse import mybir
    from concourse.bass2jax import (
        _bass_exec_p,
        install_neuronx_cc_hook,
        partition_id_tensor,
    )

    nc = _cache.get("nc")
    if nc is None:
        nc = _cache["nc"] = build_program()

    install_neuronx_cc_hook()

    partition_name = nc.partition_id_tensor.name if nc.partition_id_tensor else None
    in_names, out_names, out_avals = [], [], []
    for alloc in nc.m.functions[0].allocations:
        if not isinstance(alloc, mybir.MemoryLocationSet):
            continue
        name = alloc.memorylocations[0].name
        if alloc.kind == "ExternalInput":
            if name != partition_name:
                in_names.append(name)
        elif alloc.kind == "ExternalOutput":
            out_names.append(name)
            out_avals.append(
                jax.core.ShapedArray(tuple(alloc.tensor_shape), mybir.dt.np(alloc.dtype))
            )
    n_params = len(in_names)
    all_names = list(in_names) + out_names
    if partition_name is not None:
        all_names.append(partition_name)

    def _body(*args):
        # args = params + output placeholder buffers (the hook's parameter-
        # order check requires every bass_exec operand to be a jit parameter)
        operands = list(args)
        if partition_name is not None:
            operands.append(partition_id_tensor())
        outs = _bass_exec_p.bind(
            *operands,
            out_avals=tuple(out_avals),
            in_names=tuple(all_names),
            out_names=tuple(out_names),
            lowering_input_output_aliases=(),
            sim_require_finite=True,
            sim_require_nnan=True,
            nc=nc,
        )
        return tuple(outs)

    devices = jax.devices()[:N_CORES]
    assert len(devices) == N_CORES
    mesh = Mesh(np.asarray(devices), ("core",))
    n_outs = len(out_names)
    sharded = jax.jit(
        shard_map(
            _body,
            mesh=mesh,
            in_specs=(PartitionSpec("core"),) * (n_params + n_outs),
            out_specs=(PartitionSpec("core"),) * n_outs,
            check_rep=False,
        ),
        keep_unused=True,
    )
    # Output placeholder buffers: uploaded to device once, NOT donated, so
    # they stay valid and cost nothing on subsequent calls.
    zeros_dev = [
        jax.device_put(
            np.zeros((N_CORES * a.shape[0], *a.shape[1:]), a.dtype),
            NamedSharding(mesh, PartitionSpec("core")),
        )
        for a in out_avals
    ]
    _cache["runner"] = (sharded, in_names, out_names, zeros_dev)
    return _cache["runner"]


def kernel(images, e, t):
    images = np.asarray(images)
    orig_shape = images.shape

    x = _f8(images.astype(np.float32, copy=False).reshape(B, PIX))
    yy = _f8(np.asarray(e, dtype=np.float32).reshape(B, PIX))
    tt = np.ascontiguousarray(np.asarray(t, dtype=np.int32).reshape(B))

    try:
        sharded, in_names, out_names, zeros_dev = _get_runner()
        global_in = {"x": x, "y": yy, "t": tt}
        out_arrs = sharded(*[global_in[n] for n in in_names], *zeros_dev)
        full = np.asarray(out_arrs[out_names.index("out")])
    except Exception:
        # Fallback: the stock (slower, but battle-tested) execution path.
        from concourse import bass_utils

        if "nc" not in _cache:
            _cache["nc"] = build_program()
        res = bass_utils.run_bass_kernel_spmd(
            _cache["nc"], make_in_maps(images, e, t), core_ids=list(range(N_CORES))
        )
        full = np.concatenate([res.results[c]["out"] for c in range(N_CORES)], axis=0)

    return full.astype(np.float32).reshape(orig_shape)


# revision 13
# speedup vs baseline: 1.5619x; 1.0950x over previous
"""Trainium2 Bass kernel for the ConditionalDDPM forward-diffusion problem.

Computes  xt = sqrt(alpha_bar[t]) * images + sqrt(1 - alpha_bar[t]) * e
for B=65536 images of shape (1, 28, 28), t in [0, 1000).

Strategy (pure data parallel, 8 NeuronCores):
  - Shard images/e/t along batch: 8192 samples per core.
  - Pure HBM-bandwidth-bound (~358-435 GB/s/core).  The rel-err budget
    (2e-2) is exploited with reduced precision, quantized on the HOST with
    ml_dtypes (device only upconverts, exactly reproducible):
      images, e  ->  fp8 e3m4  (1 byte; measured global rel err 1.34e-2)
      out        ->  fp16      (2 bytes)
    Per-core traffic: 6.42 + 6.42 + 12.85 = 25.7MB vs 77MB in f32.
  - Per-sample scalars computed on device from t (no table gather):
    g(t) = ln(alpha_bar[t]) fitted by a degree-6 zero-intercept polynomial
    in u=(t+1)/1000 (f64 fit residual ~5e-13); a = exp(g/2), b = sqrt(1-e^g)
    in f32.  t is DMAed first on the sync queue so a/b are ready (~5us)
    before the first data tile lands.
  - Sample layout: sample s = 64*p + i lives at (partition p, unit i).
    Static SBUF tiles for x and e (no buffer recycling -> loads never wait).
  - Per unit i: scale u = a_i * x_i (ACT engine), then combine
    out = (b_i * e_i) + u in-place into the u tile (DVE).  The 2x64 ops at
    ~1.03us each make the kernel compute-bound (~66us/engine vs ~61us DMA
    stream).  GPSIMD elementwise offload was measured and REJECTED: Q7
    compute slows concurrent DVE ops 1.55x via SBUF port contention.
  - ALL stores on SYNC (HWDGE): 4-unit groups for units 0-47, per-unit
    tail stores for units 48-63.  An earlier variant with bulk stores on
    GPSIMD (SWDGE) intermittently corrupted ~2 units when the u-tile WAR
    window was tight (bufs=9); the HWDGE store path has never corrupted.
  - u-tile pool bufs=12: stores lag compute by ~12-15us, so the
    write-after-read recycle dependency (scale of group g+12 waits store
    of group g, ~25us of slack) never stalls the compute engines.
"""

import sys

if "/opt/trn_rl_repo" not in sys.path:
    sys.path.insert(0, "/opt/trn_rl_repo")

import numpy as np

B = 65536
T = 1000
BETA_1 = 1e-4
BETA_T = 0.02
N_CORES = 8
NS = B // N_CORES  # samples per core = 8192
PIX = 784
N_UNITS = NS // 128  # 64
LK = 8  # units per bulk load DMA
SK = 4  # units per bulk store DMA / u-tile group
BULK = 60  # units 0..59 in 4-unit store groups, 60..63 per-unit tail
POLY_DEG = 6

# engine split for the per-unit elementwise work: ACT does all 64 scale
# ops, DVE all 64 combines (Pool rejects TensorScalarPtr, so no gpsimd
# compute; gpsimd only triggers the bulk stores)
GP_COMBINE = frozenset()
DVE_SCALE = frozenset()

_cache = {}


def g_poly_coeffs() -> np.ndarray:
    """c[0..5] with g(u) ~= (((((c6*u + c5)*u + c4)*u + c3)*u + c2)*u + c1)*u,
    u = (t+1)/1000, g = ln(alpha_bar[t]).  Fit in f64; residual ~5e-13."""
    slope = (BETA_T - BETA_1) / (T - 1)
    betas = BETA_1 + slope * np.arange(T, dtype=np.float64)
    g_exact = np.cumsum(np.log1p(-betas))
    u = (np.arange(T, dtype=np.float64) + 1.0) / 1000.0
    A = np.stack([u**k for k in range(1, POLY_DEG + 1)], axis=1)
    c, *_ = np.linalg.lstsq(A, g_exact, rcond=None)
    return c


def _f8(arr32: np.ndarray) -> np.ndarray:
    import ml_dtypes

    return arr32.astype(ml_dtypes.float8_e3m4)


def build_program(ns: int = NS):
    """Build the per-core Bass program (same NEFF on all 8 cores)."""
    from concourse import bacc, mybir
    import concourse.tile as tile

    n_units = ns // 128
    f32 = mybir.dt.float32
    f16 = mybir.dt.float16
    f8 = mybir.dt.float8e3
    Alu = mybir.AluOpType
    Act = mybir.ActivationFunctionType
    coeffs = [float(c) for c in g_poly_coeffs()]

    nc = bacc.Bacc(
        "TRN2",
        target_bir_lowering=False,
        debug=False,
        enable_asserts=False,
        num_devices=N_CORES,
    )
    x = nc.dram_tensor("x", [ns, PIX], f8, kind="ExternalInput").ap()
    y = nc.dram_tensor("y", [ns, PIX], f8, kind="ExternalInput").ap()
    tt = nc.dram_tensor("t", [ns], mybir.dt.int32, kind="ExternalInput").ap()
    out = nc.dram_tensor("out", [ns, PIX], f16, kind="ExternalOutput").ap()

    # sample s = 64*p + i  ->  (partition p, unit i)
    x_v = x.rearrange("(p i) m -> p i m", p=128)
    y_v = y.rearrange("(p i) m -> p i m", p=128)
    o_v = out.rearrange("(p i) m -> p i m", p=128)
    t_v = tt.rearrange("(p i) -> p i", p=128)  # contiguous 256B per partition

    def scale_unit(i, out_ap, in_ap, a_t):
        if i in DVE_SCALE:
            nc.vector.tensor_scalar_mul(
                out=out_ap, in0=in_ap, scalar1=a_t[:, i : i + 1]
            )
        else:
            nc.scalar.activation(
                out=out_ap, in_=in_ap, func=Act.Copy, scale=a_t[:, i : i + 1]
            )

    def combine_unit(i, u_ap, e_ap, b_t):
        eng = nc.gpsimd if i in GP_COMBINE else nc.vector
        eng.scalar_tensor_tensor(
            out=u_ap,
            in0=e_ap,
            scalar=b_t[:, i : i + 1],
            in1=u_ap,
            op0=Alu.mult,
            op1=Alu.add,
        )

    with tile.TileContext(nc) as tc:
        with (
            tc.tile_pool(name="xs", bufs=1) as xpool,
            tc.tile_pool(name="ys", bufs=1) as ypool,
            tc.tile_pool(name="us", bufs=12) as upool,
            tc.tile_pool(name="ut", bufs=4) as utail,
            tc.tile_pool(name="singles", bufs=1) as singles,
        ):
            # ---- t load first (sync queue) so scalars are ready early ----
            ti = singles.tile([128, n_units], mybir.dt.int32)
            nc.sync.dma_start(out=ti[:], in_=t_v)

            # ---- per-sample scalars: a = exp(g/2), b = sqrt(1 - exp(g)) ----
            # u = (t + 1) / 1000   (int32 in, f32 out)
            uu = singles.tile([128, n_units], f32)
            nc.vector.tensor_scalar(
                out=uu[:], in0=ti[:], scalar1=1.0, scalar2=0.001,
                op0=Alu.add, op1=Alu.mult,
            )
            # Horner with zero intercept: h = u*c6; h = (h + c_k)*u, k=5..1
            hh = singles.tile([128, n_units], f32)
            nc.vector.tensor_scalar_mul(out=hh[:], in0=uu[:], scalar1=coeffs[5])
            for kk_ in range(POLY_DEG - 2, -1, -1):
                nc.vector.scalar_tensor_tensor(
                    out=hh[:], in0=hh[:], scalar=coeffs[kk_], in1=uu[:],
                    op0=Alu.add, op1=Alu.mult,
                )
            # a = exp(0.5*g)
            a_t = singles.tile([128, n_units], f32)
            nc.scalar.activation(out=a_t[:], in_=hh[:], func=Act.Exp, scale=0.5)
            # b = sqrt(1 - exp(g))
            b_t = singles.tile([128, n_units], f32)
            nc.scalar.activation(out=b_t[:], in_=hh[:], func=Act.Exp)
            nc.vector.tensor_scalar(
                out=b_t[:], in0=b_t[:], scalar1=1.0, scalar2=-1.0,
                op0=Alu.subtract, op1=Alu.mult,
            )
            nc.scalar.activation(out=b_t[:], in_=b_t[:], func=Act.Sqrt)

            # ---- static input tiles: loads never wait on anything ----
            x_sb = xpool.tile([128, n_units, PIX], f8)
            e_sb = ypool.tile([128, n_units, PIX], f8)
            # bulk loads only, LK units per DMA (x/e interleaved): compute
            # lags loads by tens of us, so fine-grained tail loads are
            # pointless and their extra DMAs just thrash the 8 HWDGE sem
            # lanes shared with the stores
            for c in range(n_units // LK):
                i0 = c * LK
                nc.sync.dma_start(out=x_sb[:, i0 : i0 + LK, :], in_=x_v[:, i0 : i0 + LK, :])
                nc.sync.dma_start(out=e_sb[:, i0 : i0 + LK, :], in_=y_v[:, i0 : i0 + LK, :])

            # ---- bulk compute + stores: SK-unit u tiles, stores on sync ----
            for g in range(BULK // SK):
                i0 = g * SK
                u4 = upool.tile([128, SK, PIX], f16)
                for kk in range(SK):
                    i = i0 + kk
                    scale_unit(i, u4[:, kk, :], x_sb[:, i, :], a_t)
                    combine_unit(i, u4[:, kk, :], e_sb[:, i, :], b_t)
                nc.sync.dma_start(out=o_v[:, i0 : i0 + SK, :], in_=u4[:])

            # ---- tail: last 4 units per-unit so the final store chain after
            # ---- the last DVE op is as short as possible
            for i in range(BULK, n_units):
                u1 = utail.tile([128, PIX], f16)
                scale_unit(i, u1[:], x_sb[:, i, :], a_t)
                combine_unit(i, u1[:], e_sb[:, i, :], b_t)
                nc.sync.dma_start(out=o_v[:, i, :], in_=u1[:])

    nc.compile()
    return nc


def make_in_maps(images, e, t):
    x = _f8(np.asarray(images, dtype=np.float32).reshape(B, PIX))
    yy = _f8(np.asarray(e, dtype=np.float32).reshape(B, PIX))
    tt = np.ascontiguousarray(np.asarray(t, dtype=np.int32).reshape(B))
    in_maps = []
    for c in range(N_CORES):
        sl = slice(c * NS, (c + 1) * NS)
        in_maps.append(
            {
                "x": np.ascontiguousarray(x[sl]),
                "y": np.ascontiguousarray(yy[sl]),
                "t": np.ascontiguousarray(tt[sl]),
            }
        )
    return in_maps


def _get_runner():
    """Build (once) a jitted shard_map callable over the 8 cores.

    Mirrors concourse.bass2jax.run_bass_via_pjrt, but caches the compiled
    executable so repeated kernel() calls skip retracing/recompiling, and
    keeps the output placeholder buffers resident on device.
    """
    if "runner" in _cache:
        return _cache["runner"]

    import jax
    from jax.sharding import Mesh, PartitionSpec, NamedSharding
    from jax.experimental.shard_map import shard_map
    from concourse import mybir
    from concourse.bass2jax import (
        _bass_exec_p,
        install_neuronx_cc_hook,
        partition_id_tensor,
    )

    nc = _cache.get("nc")
    if nc is None:
        nc = _cache["nc"] = build_program()

    install_neuronx_cc_hook()

    partition_name = nc.partition_id_tensor.name if nc.partition_id_tensor else None
    in_names, out_names, out_avals = [], [], []
    for alloc in nc.m.functions[0].allocations:
        if not isinstance(alloc, mybir.MemoryLocationSet):
            continue
        name = alloc.memorylocations[0].name
        if alloc.kind == "ExternalInput":
            if name != partition_name:
                in_names.append(name)
        elif alloc.kind == "ExternalOutput":
            out_names.append(name)
            out_avals.append(
                jax.core.ShapedArray(tuple(alloc.tensor_shape), mybir.dt.np(alloc.dtype))
            )
    n_params = len(in_names)
    all_names = list(in_names) + out_names
    if partition_name is not None:
        all_names.append(partition_name)

    def _body(*args):
        # args = params + output placeholder buffers (the hook's parameter-
        # order check requires every bass_exec operand to be a jit parameter)
        operands = list(args)
        if partition_name is not None:
            operands.append(partition_id_tensor())
        outs = _bass_exec_p.bind(
            *operands,
            out_avals=tuple(out_avals),
            in_names=tuple(all_names),
            out_names=tuple(out_names),
            lowering_input_output_aliases=(),
            sim_require_finite=True,
            sim_require_nnan=True,
            nc=nc,
        )
        return tuple(outs)

    devices = jax.devices()[:N_CORES]
    assert len(devices) == N_CORES
    mesh = Mesh(np.asarray(devices), ("core",))
    n_outs = len(out_names)
    sharded = jax.jit(
        shard_map(
            _body,
            mesh=mesh,
            in_specs=(PartitionSpec("core"),) * (n_params + n_outs),
            out_specs=(PartitionSpec("core"),) * n_outs,
            check_rep=False,
        ),
        keep_unused=True,
    )
    # Output placeholder buffers: uploaded to device once, NOT donated, so
    # they stay valid and cost nothing on subsequent calls.
    zeros_dev = [
        jax.device_put(
            np.zeros((N_CORES * a.shape[0], *a.shape[1:]), a.dtype),
            NamedSharding(mesh, PartitionSpec("core")),
        )
        for a in out_avals
    ]
    _cache["runner"] = (sharded, in_names, out_names, zeros_dev)
    return _cache["runner"]


def kernel(images, e, t):
    images = np.asarray(images)
    orig_shape = images.shape

    x = _f8(images.astype(np.float32, copy=False).reshape(B, PIX))
    yy = _f8(np.asarray(e, dtype=np.float32).reshape(B, PIX))
    tt = np.ascontiguousarray(np.asarray(t, dtype=np.int32).reshape(B))

    try:
        sharded, in_names, out_names, zeros_dev = _get_runner()
        global_in = {"x": x, "y": yy, "t": tt}
        out_arrs = sharded(*[global_in[n] for n in in_names], *zeros_dev)
        full = np.asarray(out_arrs[out_names.index("out")])
    except Exception:
        # Fallback: the stock (slower, but battle-tested) execution path.
        from concourse import bass_utils

        if "nc" not in _cache:
            _cache["nc"] = build_program()
        res = bass_utils.run_bass_kernel_spmd(
            _cache["nc"], make_in_maps(images, e, t), core_ids=list(range(N_CORES))
        )
        full = np.concatenate([res.results[c]["out"] for c in range(N_CORES)], axis=0)

    return full.astype(np.float32).reshape(orig_shape)


# revision 16
# speedup vs baseline: 1.6232x; 1.0392x over previous
"""Trainium2 Bass kernel for the ConditionalDDPM forward-diffusion problem.

Computes  xt = sqrt(alpha_bar[t]) * images + sqrt(1 - alpha_bar[t]) * e
for B=65536 images of shape (1, 28, 28), t in [0, 1000).

Strategy (pure data parallel, 8 NeuronCores):
  - Shard images/e/t along batch: 8192 samples per core.
  - Pure HBM-bandwidth-bound (~358-435 GB/s/core).  The rel-err budget
    (2e-2) is exploited with reduced precision, quantized on the HOST with
    ml_dtypes (device only upconverts, exactly reproducible):
      images, e  ->  fp8 e3m4  (1 byte; measured global rel err 1.34e-2)
      out        ->  fp16      (2 bytes)
    Per-core traffic: 6.42 + 6.42 + 12.85 = 25.7MB vs 77MB in f32.
  - Per-sample scalars computed on device from t (no table gather):
    g(t) = ln(alpha_bar[t]) fitted by a degree-6 zero-intercept polynomial
    in u=(t+1)/1000 (f64 fit residual ~5e-13); a = exp(g/2), b = sqrt(1-e^g)
    in f32.  t is DMAed first on the sync queue so a/b are ready (~5us)
    before the first data tile lands.
  - Sample layout: sample s = 64*p + i lives at (partition p, unit i).
    Static SBUF tiles for x and e (no buffer recycling -> loads never wait).
  - Per unit i: scale u = a_i * x_i (ACT engine), then combine
    out = (b_i * e_i) + u in-place into the u tile (DVE).  The 2x64 ops at
    ~1.03us each make the kernel compute-bound (~66us/engine vs ~61us DMA
    stream).  GPSIMD elementwise offload was measured and REJECTED: Q7
    compute slows concurrent DVE ops 1.55x via SBUF port contention.
  - ALL DMA on SYNC (HWDGE): loads only as 8-unit chunks (fine-grained
    tail loads are pointless once compute-bound, and extra DMAs thrash the
    8 HWDGE sem lanes shared with stores); stores as 4-unit groups for
    units 0-59, per-unit for 60-63 so the post-compute store chain is
    short.  An earlier variant with bulk stores on GPSIMD (SWDGE)
    intermittently corrupted ~2 units when the u-tile WAR window was
    tight (bufs=9); the HWDGE store path has never corrupted (9/9 calls
    across 3 runs).
  - u-tile pool bufs=12: stores lag compute by ~12-15us, so the
    write-after-read recycle dependency (scale of group g+12 waits store
    of group g, ~25us of slack) never stalls the compute engines.
"""

import sys

if "/opt/trn_rl_repo" not in sys.path:
    sys.path.insert(0, "/opt/trn_rl_repo")

import numpy as np

B = 65536
T = 1000
BETA_1 = 1e-4
BETA_T = 0.02
N_CORES = 8
NS = B // N_CORES  # samples per core = 8192
PIX = 784
N_UNITS = NS // 128  # 64
LK = 8  # units per bulk load DMA
SK = 4  # units per bulk store DMA / u-tile group
BULK = 60  # units 0..59 in 4-unit store groups, 60..63 per-unit tail
POLY_DEG = 6

# engine split for the per-unit elementwise work: ACT does all 64 scale
# ops, DVE all 64 combines (Pool rejects TensorScalarPtr, and measured
# gpsimd elementwise slows concurrent DVE 1.55x; gpsimd stays idle)
GP_COMBINE = frozenset()
DVE_SCALE = frozenset()

_cache = {}


def g_poly_coeffs() -> np.ndarray:
    """c[0..5] with g(u) ~= (((((c6*u + c5)*u + c4)*u + c3)*u + c2)*u + c1)*u,
    u = (t+1)/1000, g = ln(alpha_bar[t]).  Fit in f64; residual ~5e-13."""
    slope = (BETA_T - BETA_1) / (T - 1)
    betas = BETA_1 + slope * np.arange(T, dtype=np.float64)
    g_exact = np.cumsum(np.log1p(-betas))
    u = (np.arange(T, dtype=np.float64) + 1.0) / 1000.0
    A = np.stack([u**k for k in range(1, POLY_DEG + 1)], axis=1)
    c, *_ = np.linalg.lstsq(A, g_exact, rcond=None)
    return c


def _f8(arr32: np.ndarray) -> np.ndarray:
    import ml_dtypes

    return arr32.astype(ml_dtypes.float8_e3m4)


N_DBL = 28  # same-t double columns (cols 0..55); guaranteed feasible for any
            # t: pairs >= (NS - 1000)/2 = 3596 >= 128*28 = 3584 by pigeonhole


def _pair_perm(t_shard: np.ndarray) -> np.ndarray:
    """assign original row indices to device slots (p, i) such that columns
    2k and 2k+1 (k < N_DBL) hold equal-t samples in every partition row.
    Returns perm[8192]: device slot 64*p + i <- original row perm[64*p+i]."""
    order = np.argsort(t_shard, kind="stable")
    ts = np.asarray(t_shard)[order]
    cut = np.nonzero(np.diff(ts))[0] + 1
    pairs, singles = [], []
    for r in np.split(order, cut):
        n2 = len(r) // 2
        if n2:
            pairs.append(r[: 2 * n2].reshape(n2, 2))
        if len(r) % 2:
            singles.append(r[-1:])
    pairs = np.concatenate(pairs, axis=0)
    need = 128 * N_DBL
    assert len(pairs) >= need, (len(pairs), need)
    if len(pairs) > need:
        singles.append(pairs[need:].ravel())
        pairs = pairs[:need]
    singles = np.concatenate(singles)
    assert singles.size == 128 * (64 - 2 * N_DBL), singles.size
    assign = np.empty((128, 64), dtype=np.int64)
    pa = pairs.reshape(128, N_DBL, 2)
    assign[:, 0 : 2 * N_DBL : 2] = pa[:, :, 0]
    assign[:, 1 : 2 * N_DBL : 2] = pa[:, :, 1]
    assign[:, 2 * N_DBL :] = singles.reshape(128, 64 - 2 * N_DBL)
    return assign.reshape(-1)


def _permute_shards(x, yy, tt):
    """Apply the per-core pairing permutation; returns (x, yy, tt, perms).
    Copies any read-only input (e.g. a no-copy view of a jax buffer)."""
    if not x.flags.writeable:
        x = x.copy()
    if not yy.flags.writeable:
        yy = yy.copy()
    if not tt.flags.writeable:
        tt = tt.copy()
    perms = []
    for c in range(N_CORES):
        sl = slice(c * NS, (c + 1) * NS)
        perm = _pair_perm(tt[sl])
        perms.append(perm)
        x[sl] = x[sl][perm]
        yy[sl] = yy[sl][perm]
        tt[sl] = tt[sl][perm]
    return x, yy, tt, perms


def _unpermute_out(full, perms):
    """Invert the per-core permutation on the device output rows."""
    out = np.empty_like(full)
    for c, perm in enumerate(perms):
        sl = slice(c * NS, (c + 1) * NS)
        blk = np.empty_like(full[sl])
        blk[perm] = full[sl]
        out[sl] = blk
    return out


def build_program(ns: int = NS):
    """Build the per-core Bass program (same NEFF on all 8 cores)."""
    from concourse import bacc, mybir
    import concourse.tile as tile

    n_units = ns // 128
    f32 = mybir.dt.float32
    f16 = mybir.dt.float16
    f8 = mybir.dt.float8e3
    Alu = mybir.AluOpType
    Act = mybir.ActivationFunctionType
    coeffs = [float(c) for c in g_poly_coeffs()]

    nc = bacc.Bacc(
        "TRN2",
        target_bir_lowering=False,
        debug=False,
        enable_asserts=False,
        num_devices=N_CORES,
    )
    x = nc.dram_tensor("x", [ns, PIX], f8, kind="ExternalInput").ap()
    y = nc.dram_tensor("y", [ns, PIX], f8, kind="ExternalInput").ap()
    tt = nc.dram_tensor("t", [ns], mybir.dt.int32, kind="ExternalInput").ap()
    out = nc.dram_tensor("out", [ns, PIX], f16, kind="ExternalOutput").ap()

    # sample s = 64*p + i  ->  (partition p, unit i)
    x_v = x.rearrange("(p i) m -> p i m", p=128)
    y_v = y.rearrange("(p i) m -> p i m", p=128)
    o_v = out.rearrange("(p i) m -> p i m", p=128)
    t_v = tt.rearrange("(p i) -> p i", p=128)  # contiguous 256B per partition

    def scale_unit(i, out_ap, in_ap, a_t):
        if i in DVE_SCALE:
            nc.vector.tensor_scalar_mul(
                out=out_ap, in0=in_ap, scalar1=a_t[:, i : i + 1]
            )
        else:
            nc.scalar.activation(
                out=out_ap, in_=in_ap, func=Act.Copy, scale=a_t[:, i : i + 1]
            )

    def combine_unit(i, u_ap, e_ap, b_t):
        eng = nc.gpsimd if i in GP_COMBINE else nc.vector
        eng.scalar_tensor_tensor(
            out=u_ap,
            in0=e_ap,
            scalar=b_t[:, i : i + 1],
            in1=u_ap,
            op0=Alu.mult,
            op1=Alu.add,
        )

    with tile.TileContext(nc) as tc:
        with (
            tc.tile_pool(name="xs", bufs=1) as xpool,
            tc.tile_pool(name="ys", bufs=1) as ypool,
            tc.tile_pool(name="us", bufs=12) as upool,
            tc.tile_pool(name="ut", bufs=4) as utail,
            tc.tile_pool(name="singles", bufs=1) as singles,
        ):
            # ---- t load first (sync queue) so scalars are ready early ----
            ti = singles.tile([128, n_units], mybir.dt.int32)
            nc.sync.dma_start(out=ti[:], in_=t_v)

            # ---- per-sample scalars: a = exp(g/2), b = sqrt(1 - exp(g)) ----
            # u = (t + 1) / 1000   (int32 in, f32 out)
            uu = singles.tile([128, n_units], f32)
            nc.vector.tensor_scalar(
                out=uu[:], in0=ti[:], scalar1=1.0, scalar2=0.001,
                op0=Alu.add, op1=Alu.mult,
            )
            # Horner with zero intercept: h = u*c6; h = (h + c_k)*u, k=5..1
            hh = singles.tile([128, n_units], f32)
            nc.vector.tensor_scalar_mul(out=hh[:], in0=uu[:], scalar1=coeffs[5])
            for kk_ in range(POLY_DEG - 2, -1, -1):
                nc.vector.scalar_tensor_tensor(
                    out=hh[:], in0=hh[:], scalar=coeffs[kk_], in1=uu[:],
                    op0=Alu.add, op1=Alu.mult,
                )
            # a = exp(0.5*g)
            a_t = singles.tile([128, n_units], f32)
            nc.scalar.activation(out=a_t[:], in_=hh[:], func=Act.Exp, scale=0.5)
            # b = sqrt(1 - exp(g))
            b_t = singles.tile([128, n_units], f32)
            nc.scalar.activation(out=b_t[:], in_=hh[:], func=Act.Exp)
            nc.vector.tensor_scalar(
                out=b_t[:], in0=b_t[:], scalar1=1.0, scalar2=-1.0,
                op0=Alu.subtract, op1=Alu.mult,
            )
            nc.scalar.activation(out=b_t[:], in_=b_t[:], func=Act.Sqrt)

            # ---- static input tiles: loads never wait on anything ----
            x_sb = xpool.tile([128, n_units, PIX], f8)
            e_sb = ypool.tile([128, n_units, PIX], f8)
            # bulk loads only, LK units per DMA (x/e interleaved): compute
            # lags loads by tens of us, so fine-grained tail loads are
            # pointless and their extra DMAs just thrash the 8 HWDGE sem
            # lanes shared with the stores
            for c in range(n_units // LK):
                i0 = c * LK
                nc.sync.dma_start(out=x_sb[:, i0 : i0 + LK, :], in_=x_v[:, i0 : i0 + LK, :])
                nc.sync.dma_start(out=e_sb[:, i0 : i0 + LK, :], in_=y_v[:, i0 : i0 + LK, :])

            # ---- bulk compute + stores: SK-unit u tiles, stores on sync.
            # ---- Columns 0..2*N_DBL-1 are same-t PAIRS (host permutation):
            # ---- one op covers two columns, halving per-op overhead.
            for g in range(BULK // SK):
                i0 = g * SK
                u4 = upool.tile([128, SK, PIX], f16)
                if i0 + SK <= 2 * N_DBL:
                    for dd in range(SK // 2):
                        j = i0 + 2 * dd
                        nc.scalar.activation(
                            out=u4[:, 2 * dd : 2 * dd + 2, :],
                            in_=x_sb[:, j : j + 2, :],
                            func=Act.Copy,
                            scale=a_t[:, j : j + 1],
                        )
                        nc.vector.scalar_tensor_tensor(
                            out=u4[:, 2 * dd : 2 * dd + 2, :],
                            in0=e_sb[:, j : j + 2, :],
                            scalar=b_t[:, j : j + 1],
                            in1=u4[:, 2 * dd : 2 * dd + 2, :],
                            op0=Alu.mult,
                            op1=Alu.add,
                        )
                else:
                    for kk in range(SK):
                        i = i0 + kk
                        scale_unit(i, u4[:, kk, :], x_sb[:, i, :], a_t)
                        combine_unit(i, u4[:, kk, :], e_sb[:, i, :], b_t)
                nc.sync.dma_start(out=o_v[:, i0 : i0 + SK, :], in_=u4[:])

            # ---- tail: last 4 units per-unit so the final store chain after
            # ---- the last DVE op is as short as possible
            for i in range(BULK, n_units):
                u1 = utail.tile([128, PIX], f16)
                scale_unit(i, u1[:], x_sb[:, i, :], a_t)
                combine_unit(i, u1[:], e_sb[:, i, :], b_t)
                nc.sync.dma_start(out=o_v[:, i, :], in_=u1[:])

    nc.compile()
    return nc


def make_in_maps(images, e, t):
    x = _f8(np.asarray(images, dtype=np.float32).reshape(B, PIX))
    yy = _f8(np.asarray(e, dtype=np.float32).reshape(B, PIX))
    tt = np.ascontiguousarray(np.asarray(t, dtype=np.int32).reshape(B))
    x, yy, tt, _cache["perms"] = _permute_shards(x, yy, tt)
    in_maps = []
    for c in range(N_CORES):
        sl = slice(c * NS, (c + 1) * NS)
        in_maps.append(
            {
                "x": np.ascontiguousarray(x[sl]),
                "y": np.ascontiguousarray(yy[sl]),
                "t": np.ascontiguousarray(tt[sl]),
            }
        )
    return in_maps


def _get_runner():
    """Build (once) a jitted shard_map callable over the 8 cores.

    Mirrors concourse.bass2jax.run_bass_via_pjrt, but caches the compiled
    executable so repeated kernel() calls skip retracing/recompiling, and
    keeps the output placeholder buffers resident on device.
    """
    if "runner" in _cache:
        return _cache["runner"]

    import jax
    from jax.sharding import Mesh, PartitionSpec, NamedSharding
    from jax.experimental.shard_map import shard_map
    from concourse import mybir
    from concourse.bass2jax import (
        _bass_exec_p,
        install_neuronx_cc_hook,
        partition_id_tensor,
    )

    nc = _cache.get("nc")
    if nc is None:
        nc = _cache["nc"] = build_program()

    install_neuronx_cc_hook()

    partition_name = nc.partition_id_tensor.name if nc.partition_id_tensor else None
    in_names, out_names, out_avals = [], [], []
    for alloc in nc.m.functions[0].allocations:
        if not isinstance(alloc, mybir.MemoryLocationSet):
            continue
        name = alloc.memorylocations[0].name
        if alloc.kind == "ExternalInput":
            if name != partition_name:
                in_names.append(name)
        elif alloc.kind == "ExternalOutput":
            out_names.append(name)
            out_avals.append(
                jax.core.ShapedArray(tuple(alloc.tensor_shape), mybir.dt.np(alloc.dtype))
            )
    n_params = len(in_names)
    all_names = list(in_names) + out_names
    if partition_name is not None:
        all_names.append(partition_name)

    def _body(*args):
        # args = params + output placeholder buffers (the hook's parameter-
        # order check requires every bass_exec operand to be a jit parameter)
        operands = list(args)
        if partition_name is not None:
            operands.append(partition_id_tensor())
        outs = _bass_exec_p.bind(
            *operands,
            out_avals=tuple(out_avals),
            in_names=tuple(all_names),
            out_names=tuple(out_names),
            lowering_input_output_aliases=(),
            sim_require_finite=True,
            sim_require_nnan=True,
            nc=nc,
        )
        return tuple(outs)

    devices = jax.devices()[:N_CORES]
    assert len(devices) == N_CORES
    mesh = Mesh(np.asarray(devices), ("core",))
    n_outs = len(out_names)
    sharded = jax.jit(
        shard_map(
            _body,
            mesh=mesh,
            in_specs=(PartitionSpec("core"),) * (n_params + n_outs),
            out_specs=(PartitionSpec("core"),) * n_outs,
            check_rep=False,
        ),
        keep_unused=True,
    )
    # Output placeholder buffers: uploaded to device once, NOT donated, so
    # they stay valid and cost nothing on subsequent calls.
    zeros_dev = [
        jax.device_put(
            np.zeros((N_CORES * a.shape[0], *a.shape[1:]), a.dtype),
            NamedSharding(mesh, PartitionSpec("core")),
        )
        for a in out_avals
    ]
    _cache["runner"] = (sharded, in_names, out_names, zeros_dev)
    return _cache["runner"]


def kernel(images, e, t):
    images = np.asarray(images)
    orig_shape = images.shape

    x = _f8(images.astype(np.float32, copy=False).reshape(B, PIX))
    yy = _f8(np.asarray(e, dtype=np.float32).reshape(B, PIX))
    tt = np.ascontiguousarray(np.asarray(t, dtype=np.int32).reshape(B))
    x, yy, tt, perms = _permute_shards(x, yy, tt)

    try:
        sharded, in_names, out_names, zeros_dev = _get_runner()
        global_in = {"x": x, "y": yy, "t": tt}
        out_arrs = sharded(*[global_in[n] for n in in_names], *zeros_dev)
        full = np.asarray(out_arrs[out_names.index("out")])
    except Exception:
        # Fallback: the stock (slower, but battle-tested) execution path.
        from concourse import bass_utils

        if "nc" not in _cache:
            _cache["nc"] = build_program()
        res = bass_utils.run_bass_kernel_spmd(
            _cache["nc"], make_in_maps(images, e, t), core_ids=list(range(N_CORES))
        )
        full = np.concatenate([res.results[c]["out"] for c in range(N_CORES)], axis=0)

    return _unpermute_out(full, perms).astype(np.float32).reshape(orig_shape)


# revision 18
# speedup vs baseline: 1.6274x; 1.0026x over previous
"""Trainium2 Bass kernel for the ConditionalDDPM forward-diffusion problem.

Computes  xt = sqrt(alpha_bar[t]) * images + sqrt(1 - alpha_bar[t]) * e
for B=65536 images of shape (1, 28, 28), t in [0, 1000).

Strategy (pure data parallel, 8 NeuronCores):
  - Shard images/e/t along batch: 8192 samples per core.
  - Pure HBM-bandwidth-bound (~358-435 GB/s/core).  The rel-err budget
    (2e-2) is exploited with reduced precision, quantized on the HOST with
    ml_dtypes (device only upconverts, exactly reproducible):
      images, e  ->  fp8 e3m4  (1 byte; measured global rel err 1.34e-2)
      out        ->  fp16      (2 bytes)
    Per-core traffic: 6.42 + 6.42 + 12.85 = 25.7MB vs 77MB in f32.
  - Per-sample scalars computed on device from t (no table gather):
    g(t) = ln(alpha_bar[t]) fitted by a degree-6 zero-intercept polynomial
    in u=(t+1)/1000 (f64 fit residual ~5e-13); a = exp(g/2), b = sqrt(1-e^g)
    in f32.  t is DMAed first on the sync queue so a/b are ready (~5us)
    before the first data tile lands.
  - Sample layout: sample s = 64*p + i lives at (partition p, unit i).
    Static SBUF tiles for x and e (no buffer recycling -> loads never wait).
  - Per column i: scale u = a_i * x_i (ACT engine), then combine
    out = (b_i * e_i) + u in-place into the u tile (DVE); compute-bound
    (~60us/engine vs ~61us DMA stream).  To cut per-op overhead the HOST
    permutes each core's samples so columns 2k/2k+1 (k < 28) hold
    equal-t samples in every partition row: one op then covers TWO
    columns with a single per-partition scalar (feasible for any t by
    pigeonhole: pairs >= (8192-1000)/2 = 3596 >= 3584).  The output is
    un-permuted on the host.  GPSIMD elementwise offload was measured
    and REJECTED: Q7 compute slows concurrent DVE ops 1.55x via SBUF
    port contention.
  - ALL DMA on SYNC (HWDGE): loads only as 8-unit chunks (fine-grained
    tail loads are pointless once compute-bound, and extra DMAs thrash the
    8 HWDGE sem lanes shared with stores); stores as 4-unit groups for
    units 0-59, per-unit for 60-63 so the post-compute store chain is
    short.  An earlier variant with bulk stores on GPSIMD (SWDGE)
    intermittently corrupted ~2 units when the u-tile WAR window was
    tight (bufs=9); the HWDGE store path has never corrupted (9/9 calls
    across 3 runs).
  - u-tile pool bufs=12: stores lag compute by ~12-15us, so the
    write-after-read recycle dependency (scale of group g+12 waits store
    of group g, ~25us of slack) never stalls the compute engines.
"""

import sys

if "/opt/trn_rl_repo" not in sys.path:
    sys.path.insert(0, "/opt/trn_rl_repo")

import numpy as np

B = 65536
T = 1000
BETA_1 = 1e-4
BETA_T = 0.02
N_CORES = 8
NS = B // N_CORES  # samples per core = 8192
PIX = 784
N_UNITS = NS // 128  # 64
LK = 8  # units per bulk load DMA
SK = 4  # units per bulk store DMA / u-tile group
BULK = 60  # units 0..59 in 4-unit store groups, 60..63 per-unit tail
POLY_DEG = 6

# engine split for the per-unit elementwise work: ACT does all 64 scale
# ops, DVE all 64 combines (Pool rejects TensorScalarPtr, and measured
# gpsimd elementwise slows concurrent DVE 1.55x; gpsimd stays idle)
GP_COMBINE = frozenset()
DVE_SCALE = frozenset()

_cache = {}


def g_poly_coeffs() -> np.ndarray:
    """c[0..5] with g(u) ~= (((((c6*u + c5)*u + c4)*u + c3)*u + c2)*u + c1)*u,
    u = (t+1)/1000, g = ln(alpha_bar[t]).  Fit in f64; residual ~5e-13."""
    slope = (BETA_T - BETA_1) / (T - 1)
    betas = BETA_1 + slope * np.arange(T, dtype=np.float64)
    g_exact = np.cumsum(np.log1p(-betas))
    u = (np.arange(T, dtype=np.float64) + 1.0) / 1000.0
    A = np.stack([u**k for k in range(1, POLY_DEG + 1)], axis=1)
    c, *_ = np.linalg.lstsq(A, g_exact, rcond=None)
    return c


def _f8(arr32: np.ndarray) -> np.ndarray:
    import ml_dtypes

    return arr32.astype(ml_dtypes.float8_e3m4)


# Same-t grouping: cols 0..39 are QUAD columns (one op per 4 cols), cols
# 40..55 DOUBLE columns (one op per 2 cols), cols 56..63 singles.
# Guaranteed feasible for ANY t by pigeonhole: quads >= (8192-3*1000)/4 =
# 1298 >= 1280; after removing 1280 quads, pairs >= (3072-1000)/2 = 1036
# >= 1024; singles are the exact 1024 remainder.
N_QC = 10  # quad columns-of-4 (cols 0..39)
N_DBL = 8  # double column-pairs (cols 40..55)


def _pair_perm(t_shard: np.ndarray) -> np.ndarray:
    """assign original row indices to device slots (p, i) such that cols
    4k..4k+3 (k < N_QC) and col pairs beyond share t per partition row.
    Returns perm[8192]: device slot 64*p + i <- original row perm[64*p+i]."""
    order = np.argsort(t_shard, kind="stable")
    ts = np.asarray(t_shard)[order]
    cut = np.nonzero(np.diff(ts))[0] + 1
    quads, rest = [], []
    for r in np.split(order, cut):
        n4 = len(r) // 4
        if n4:
            quads.append(r[: 4 * n4].reshape(n4, 4))
        if len(r) % 4:
            rest.append(r[4 * n4 :])
    quads = np.concatenate(quads, axis=0)
    need_q = 128 * N_QC
    assert len(quads) >= need_q, (len(quads), need_q)
    pairs, singles = [], []
    if len(quads) > need_q:
        pairs.append(quads[need_q:].reshape(-1, 2))
        quads = quads[:need_q]
    for r in rest:
        n2 = len(r) // 2
        if n2:
            pairs.append(r[: 2 * n2].reshape(n2, 2))
        if len(r) % 2:
            singles.append(r[-1:])
    pairs = np.concatenate(pairs, axis=0)
    need_p = 128 * N_DBL
    assert len(pairs) >= need_p, (len(pairs), need_p)
    if len(pairs) > need_p:
        singles.append(pairs[need_p:].ravel())
        pairs = pairs[:need_p]
    singles = np.concatenate(singles)
    n_sc = 64 - 4 * N_QC - 2 * N_DBL
    assert singles.size == 128 * n_sc, singles.size
    assign = np.empty((128, 64), dtype=np.int64)
    assign[:, : 4 * N_QC] = quads.reshape(128, 4 * N_QC)
    assign[:, 4 * N_QC : 4 * N_QC + 2 * N_DBL] = pairs.reshape(128, 2 * N_DBL)
    assign[:, 4 * N_QC + 2 * N_DBL :] = singles.reshape(128, n_sc)
    return assign.reshape(-1)


def _permute_shards(x, yy, tt):
    """Apply the per-core pairing permutation; returns (x, yy, tt, perms).
    Copies any read-only input (e.g. a no-copy view of a jax buffer)."""
    if not x.flags.writeable:
        x = x.copy()
    if not yy.flags.writeable:
        yy = yy.copy()
    if not tt.flags.writeable:
        tt = tt.copy()
    perms = []
    for c in range(N_CORES):
        sl = slice(c * NS, (c + 1) * NS)
        perm = _pair_perm(tt[sl])
        perms.append(perm)
        x[sl] = x[sl][perm]
        yy[sl] = yy[sl][perm]
        tt[sl] = tt[sl][perm]
    return x, yy, tt, perms


def _unpermute_out(full, perms):
    """Invert the per-core permutation on the device output rows."""
    out = np.empty_like(full)
    for c, perm in enumerate(perms):
        sl = slice(c * NS, (c + 1) * NS)
        blk = np.empty_like(full[sl])
        blk[perm] = full[sl]
        out[sl] = blk
    return out


def build_program(ns: int = NS):
    """Build the per-core Bass program (same NEFF on all 8 cores)."""
    from concourse import bacc, mybir
    import concourse.tile as tile

    n_units = ns // 128
    f32 = mybir.dt.float32
    f16 = mybir.dt.float16
    f8 = mybir.dt.float8e3
    Alu = mybir.AluOpType
    Act = mybir.ActivationFunctionType
    coeffs = [float(c) for c in g_poly_coeffs()]

    nc = bacc.Bacc(
        "TRN2",
        target_bir_lowering=False,
        debug=False,
        enable_asserts=False,
        num_devices=N_CORES,
    )
    x = nc.dram_tensor("x", [ns, PIX], f8, kind="ExternalInput").ap()
    y = nc.dram_tensor("y", [ns, PIX], f8, kind="ExternalInput").ap()
    tt = nc.dram_tensor("t", [ns], mybir.dt.int32, kind="ExternalInput").ap()
    out = nc.dram_tensor("out", [ns, PIX], f16, kind="ExternalOutput").ap()

    # sample s = 64*p + i  ->  (partition p, unit i)
    x_v = x.rearrange("(p i) m -> p i m", p=128)
    y_v = y.rearrange("(p i) m -> p i m", p=128)
    o_v = out.rearrange("(p i) m -> p i m", p=128)
    t_v = tt.rearrange("(p i) -> p i", p=128)  # contiguous 256B per partition

    def scale_unit(i, out_ap, in_ap, a_t):
        if i in DVE_SCALE:
            nc.vector.tensor_scalar_mul(
                out=out_ap, in0=in_ap, scalar1=a_t[:, i : i + 1]
            )
        else:
            nc.scalar.activation(
                out=out_ap, in_=in_ap, func=Act.Copy, scale=a_t[:, i : i + 1]
            )

    def combine_unit(i, u_ap, e_ap, b_t):
        eng = nc.gpsimd if i in GP_COMBINE else nc.vector
        eng.scalar_tensor_tensor(
            out=u_ap,
            in0=e_ap,
            scalar=b_t[:, i : i + 1],
            in1=u_ap,
            op0=Alu.mult,
            op1=Alu.add,
        )

    with tile.TileContext(nc) as tc:
        with (
            tc.tile_pool(name="xs", bufs=1) as xpool,
            tc.tile_pool(name="ys", bufs=1) as ypool,
            tc.tile_pool(name="us", bufs=12) as upool,
            tc.tile_pool(name="ut", bufs=4) as utail,
            tc.tile_pool(name="singles", bufs=1) as singles,
        ):
            # ---- t load first (sync queue) so scalars are ready early ----
            ti = singles.tile([128, n_units], mybir.dt.int32)
            nc.sync.dma_start(out=ti[:], in_=t_v)

            # ---- per-sample scalars: a = exp(g/2), b = sqrt(1 - exp(g)) ----
            # u = (t + 1) / 1000   (int32 in, f32 out)
            uu = singles.tile([128, n_units], f32)
            nc.vector.tensor_scalar(
                out=uu[:], in0=ti[:], scalar1=1.0, scalar2=0.001,
                op0=Alu.add, op1=Alu.mult,
            )
            # Horner with zero intercept: h = u*c6; h = (h + c_k)*u, k=5..1
            hh = singles.tile([128, n_units], f32)
            nc.vector.tensor_scalar_mul(out=hh[:], in0=uu[:], scalar1=coeffs[5])
            for kk_ in range(POLY_DEG - 2, -1, -1):
                nc.vector.scalar_tensor_tensor(
                    out=hh[:], in0=hh[:], scalar=coeffs[kk_], in1=uu[:],
                    op0=Alu.add, op1=Alu.mult,
                )
            # a = exp(0.5*g)
            a_t = singles.tile([128, n_units], f32)
            nc.scalar.activation(out=a_t[:], in_=hh[:], func=Act.Exp, scale=0.5)
            # b = sqrt(1 - exp(g))
            b_t = singles.tile([128, n_units], f32)
            nc.scalar.activation(out=b_t[:], in_=hh[:], func=Act.Exp)
            nc.vector.tensor_scalar(
                out=b_t[:], in0=b_t[:], scalar1=1.0, scalar2=-1.0,
                op0=Alu.subtract, op1=Alu.mult,
            )
            nc.scalar.activation(out=b_t[:], in_=b_t[:], func=Act.Sqrt)

            # ---- static input tiles: loads never wait on anything ----
            x_sb = xpool.tile([128, n_units, PIX], f8)
            e_sb = ypool.tile([128, n_units, PIX], f8)
            # bulk loads only, LK units per DMA (x/e interleaved): compute
            # lags loads by tens of us, so fine-grained tail loads are
            # pointless and their extra DMAs just thrash the 8 HWDGE sem
            # lanes shared with the stores
            for c in range(n_units // LK):
                i0 = c * LK
                nc.sync.dma_start(out=x_sb[:, i0 : i0 + LK, :], in_=x_v[:, i0 : i0 + LK, :])
                nc.sync.dma_start(out=e_sb[:, i0 : i0 + LK, :], in_=y_v[:, i0 : i0 + LK, :])

            # ---- bulk compute + stores: SK-unit u tiles, stores on sync.
            # ---- Columns 0..2*N_DBL-1 are same-t PAIRS (host permutation):
            # ---- one op covers two columns, halving per-op overhead.
            for g in range(BULK // SK):
                i0 = g * SK
                u4 = upool.tile([128, SK, PIX], f16)
                if i0 + SK <= 4 * N_QC:
                    # quad column: one op covers all 4 same-t columns
                    nc.scalar.activation(
                        out=u4[:], in_=x_sb[:, i0 : i0 + SK, :],
                        func=Act.Copy, scale=a_t[:, i0 : i0 + 1],
                    )
                    nc.vector.scalar_tensor_tensor(
                        out=u4[:], in0=e_sb[:, i0 : i0 + SK, :],
                        scalar=b_t[:, i0 : i0 + 1], in1=u4[:],
                        op0=Alu.mult, op1=Alu.add,
                    )
                elif i0 + SK <= 4 * N_QC + 2 * N_DBL:
                    for dd in range(SK // 2):
                        j = i0 + 2 * dd
                        nc.scalar.activation(
                            out=u4[:, 2 * dd : 2 * dd + 2, :],
                            in_=x_sb[:, j : j + 2, :],
                            func=Act.Copy,
                            scale=a_t[:, j : j + 1],
                        )
                        nc.vector.scalar_tensor_tensor(
                            out=u4[:, 2 * dd : 2 * dd + 2, :],
                            in0=e_sb[:, j : j + 2, :],
                            scalar=b_t[:, j : j + 1],
                            in1=u4[:, 2 * dd : 2 * dd + 2, :],
                            op0=Alu.mult,
                            op1=Alu.add,
                        )
                else:
                    for kk in range(SK):
                        i = i0 + kk
                        scale_unit(i, u4[:, kk, :], x_sb[:, i, :], a_t)
                        combine_unit(i, u4[:, kk, :], e_sb[:, i, :], b_t)
                nc.sync.dma_start(out=o_v[:, i0 : i0 + SK, :], in_=u4[:])

            # ---- tail: last 4 units per-unit so the final store chain after
            # ---- the last DVE op is as short as possible
            for i in range(BULK, n_units):
                u1 = utail.tile([128, PIX], f16)
                scale_unit(i, u1[:], x_sb[:, i, :], a_t)
                combine_unit(i, u1[:], e_sb[:, i, :], b_t)
                nc.sync.dma_start(out=o_v[:, i, :], in_=u1[:])

    nc.compile()
    return nc


def make_in_maps(images, e, t):
    x = _f8(np.asarray(images, dtype=np.float32).reshape(B, PIX))
    yy = _f8(np.asarray(e, dtype=np.float32).reshape(B, PIX))
    tt = np.ascontiguousarray(np.asarray(t, dtype=np.int32).reshape(B))
    x, yy, tt, _cache["perms"] = _permute_shards(x, yy, tt)
    in_maps = []
    for c in range(N_CORES):
        sl = slice(c * NS, (c + 1) * NS)
        in_maps.append(
            {
                "x": np.ascontiguousarray(x[sl]),
                "y": np.ascontiguousarray(yy[sl]),
                "t": np.ascontiguousarray(tt[sl]),
            }
        )
    return in_maps


def _get_runner():
    """Build (once) a jitted shard_map callable over the 8 cores.

    Mirrors concourse.bass2jax.run_bass_via_pjrt, but caches the compiled
    executable so repeated kernel() calls skip retracing/recompiling, and
    keeps the output placeholder buffers resident on device.
    """
    if "runner" in _cache:
        return _cache["runner"]

    import jax
    from jax.sharding import Mesh, PartitionSpec, NamedSharding
    from jax.experimental.shard_map import shard_map
    from concourse import mybir
    from concourse.bass2jax import (
        _bass_exec_p,
        install_neuronx_cc_hook,
        partition_id_tensor,
    )

    nc = _cache.get("nc")
    if nc is None:
        nc = _cache["nc"] = build_program()

    install_neuronx_cc_hook()

    partition_name = nc.partition_id_tensor.name if nc.partition_id_tensor else None
    in_names, out_names, out_avals = [], [], []
    for alloc in nc.m.functions[0].allocations:
        if not isinstance(alloc, mybir.MemoryLocationSet):
            continue
        name = alloc.memorylocations[0].name
        if alloc.kind == "ExternalInput":
            if name != partition_name:
                in_names.append(name)
        elif alloc.kind == "ExternalOutput":
            out_names.append(name)
            out_avals.append(
                jax.core.ShapedArray(tuple(alloc.tensor_shape), mybir.dt.np(alloc.dtype))
            )
    n_params = len(in_names)
    all_names = list(in_names) + out_names
    if partition_name is not None:
        all_names.append(partition_name)

    def _body(*args):
        # args = params + output placeholder buffers (the hook's parameter-
        # order check requires every bass_exec operand to be a jit parameter)
        operands = list(args)
        if partition_name is not None:
            operands.append(partition_id_tensor())
        outs = _bass_exec_p.bind(
            *operands,
            out_avals=tuple(out_avals),
            in_names=tuple(all_names),
            out_names=tuple(out_names),
            lowering_input_output_aliases=(),
            sim_require_finite=True,
            sim_require_nnan=True,
            nc=nc,
        )
        return tuple(outs)

    devices = jax.devices()[:N_CORES]
    assert len(devices) == N_CORES
    mesh = Mesh(np.asarray(devices), ("core",))
    n_outs = len(out_names)
    sharded = jax.jit(
        shard_map(
            _body,
            mesh=mesh,
            in_specs=(PartitionSpec("core"),) * (n_params + n_outs),
            out_specs=(PartitionSpec("core"),) * n_outs,
            check_rep=False,
        ),
        keep_unused=True,
    )
    # Output placeholder buffers: uploaded to device once, NOT donated, so
    # they stay valid and cost nothing on subsequent calls.
    zeros_dev = [
        jax.device_put(
            np.zeros((N_CORES * a.shape[0], *a.shape[1:]), a.dtype),
            NamedSharding(mesh, PartitionSpec("core")),
        )
        for a in out_avals
    ]
    _cache["runner"] = (sharded, in_names, out_names, zeros_dev)
    return _cache["runner"]


def kernel(images, e, t):
    images = np.asarray(images)
    orig_shape = images.shape

    x = _f8(images.astype(np.float32, copy=False).reshape(B, PIX))
    yy = _f8(np.asarray(e, dtype=np.float32).reshape(B, PIX))
    tt = np.ascontiguousarray(np.asarray(t, dtype=np.int32).reshape(B))
    x, yy, tt, perms = _permute_shards(x, yy, tt)

    try:
        sharded, in_names, out_names, zeros_dev = _get_runner()
        global_in = {"x": x, "y": yy, "t": tt}
        out_arrs = sharded(*[global_in[n] for n in in_names], *zeros_dev)
        full = np.asarray(out_arrs[out_names.index("out")])
    except Exception:
        # Fallback: the stock (slower, but battle-tested) execution path.
        from concourse import bass_utils

        if "nc" not in _cache:
            _cache["nc"] = build_program()
        res = bass_utils.run_bass_kernel_spmd(
            _cache["nc"], make_in_maps(images, e, t), core_ids=list(range(N_CORES))
        )
        full = np.concatenate([res.results[c]["out"] for c in range(N_CORES)], axis=0)

    return _unpermute_out(full, perms).astype(np.float32).reshape(orig_shape)
